# revision 1
# baseline (speedup 1.0000x reference)
"""Trainium2 Bass kernel for LlamaAttention (B=1, S=2048, H=4096, 32 heads).

Sharding: tensor-parallel over heads. 8 cores x 4 heads. Each core:
  - computes Q^T, K^T (head-dim on partitions) and V (natural layout) for its
    4 heads from the full hidden states,
  - applies RoPE to Q^T/K^T (rotate-half = partition shift via SBUF DMA),
  - causal attention in transposed layout: scores^T = K_h @ Q_h^T per
    (j-tile, i-chunk) block, skipping fully-masked blocks; softmax without
    max-subtraction (scores are O(10) here); column sums via ones-matmul;
  - attn_out^T = V^T-contraction accumulated in PSUM, normalized by 1/sum,
  - partial o_proj: po_c = Wo[rows_c]^T @ attnT_c  (4096 x 2048).
Host sums the 8 partials and transposes -> (1, 2048, 4096). No collectives.

Matmuls run as float32r (fp32 data, 1 cycle/row at N>=512 moving dim).
"""

import os
import sys

if "/opt/trn_rl_repo" not in sys.path:
    sys.path.insert(0, "/opt/trn_rl_repo")

import numpy as np

from concourse import bacc, mybir, tile
from concourse import bass
from concourse.bass_utils import run_bass_kernel_spmd

F32 = mybir.dt.float32
F32R = mybir.dt.float32r
EXPF = mybir.ActivationFunctionType.Exp
ADD = mybir.AluOpType.add
MULT = mybir.AluOpType.mult

N_CORES = 8
S = 2048
H = 4096
N_HEADS = 32
D = 128                      # head dim
HPC = N_HEADS // N_CORES     # heads per core = 4
HC = HPC * D                 # per-core hidden slice = 512
CH = 512                     # seq chunk width
NCH = S // CH                # 4 chunks
KT_TILES = H // 128          # 32 contraction tiles for projections
SJT = S // 128               # 16 seq j-tiles
ROPE_BASE = 10000.0
NEG = -1.0e9

last_exec_time_ns = None


def _r(x):
    return np.ascontiguousarray(x, dtype=np.float32)


def _build(causal: bool, proj_bf16: bool = False):
    PDT = mybir.dt.bfloat16 if proj_bf16 else F32R
    nc = bacc.Bacc("TRN2", target_bir_lowering=False, debug=False,
                   num_devices=N_CORES)
    hT = nc.dram_tensor("hT", [H, S], PDT, kind="ExternalInput")
    wq = nc.dram_tensor("wq", [H, HC], PDT, kind="ExternalInput")
    wk = nc.dram_tensor("wk", [H, HC], PDT, kind="ExternalInput")
    wv = nc.dram_tensor("wv", [H, HC], PDT, kind="ExternalInput")
    wo = nc.dram_tensor("wo", [HC, H], F32R, kind="ExternalInput")
    cosT = nc.dram_tensor("cosT", [D, S], F32, kind="ExternalInput")
    sinTs = nc.dram_tensor("sinTs", [D, S], F32, kind="ExternalInput")
    if causal:
        mpat = nc.dram_tensor("mpat", [4, 128, CH], F32, kind="ExternalInput")
    else:
        maskT = nc.dram_tensor("maskT", [S, S], F32, kind="ExternalInput")
    po = nc.dram_tensor("po", [H, S], F32, kind="ExternalOutput")

    def mm(out, lhsT, rhs, start, stop):
        nc.tensor.matmul(out, lhsT, rhs, start=start, stop=stop)

    atb = nc.dram_tensor("atb", [HC, S], F32R)   # attn out^T spill

    from contextlib import ExitStack
    with tile.TileContext(nc) as tc:
        es_res = ExitStack()
        kt_pool = es_res.enter_context(tc.tile_pool(name="kt", bufs=HPC))
        v_pool = es_res.enter_context(tc.tile_pool(name="v", bufs=SJT))
        KT = [kt_pool.tile([128, S], F32R, tag="kt", name=f"KT{i}")
              for i in range(HPC)]
        V = [v_pool.tile([128, HC], F32R, tag="v", name=f"V{i}")
             for i in range(SJT)]

        with tc.tile_pool(name="qtc", bufs=8) as qtp, \
             tc.tile_pool(name="ht", bufs=33) as htp, \
             tc.tile_pool(name="wst", bufs=6) as wp, \
             tc.tile_pool(name="cs", bufs=2) as csp, \
             tc.tile_pool(name="rope", bufs=2) as rp, \
             tc.tile_pool(name="aconst", bufs=1) as cpool, \
             tc.tile_pool(name="aes", bufs=3) as esp, \
             tc.tile_pool(name="am", bufs=3 if not causal else 1) as mpool, \
             tc.tile_pool(name="ar", bufs=3) as rpool, \
             tc.tile_pool(name="atst", bufs=3) as atsp, \
             tc.tile_pool(name="mainps", bufs=6, space="PSUM") as psp:
            ones_col32 = cpool.tile([128, 1], F32, tag="oc32")
            ones_row32 = cpool.tile([1, 128], F32, tag="or32")
            nc.vector.memset(ones_col32[:], 1.0)
            nc.vector.memset(ones_row32[:], 1.0)
            ones_col = cpool.tile([128, 1], F32R, tag="oc")
            ones_row = cpool.tile([1, 128], F32R, tag="or")
            nc.vector.tensor_copy(ones_col[:], ones_col32[:])
            nc.vector.tensor_copy(ones_row[:], ones_row32[:])
            mtiles = []
            if causal:
                for p in range(4):
                    mt = cpool.tile([128, CH], F32, tag=f"mp{p}", name=f"mt{p}")
                    nc.sync.dma_start(out=mt[:], in_=mpat[p])
                    mtiles.append(mt)

            def rope_evict(ps, dst_ap, cosc, sinc):
                # dst = psum*cos + shift(psum)*sin_signed
                raw = rp.tile([128, CH], F32, tag="raw", name="raw")
                nc.scalar.copy(out=raw[:], in_=ps[:])
                shf = rp.tile([128, CH], F32, tag="shf", name="shf")
                nc.sync.dma_start(out=shf[0:64, :], in_=raw[64:128, :])
                nc.sync.dma_start(out=shf[64:128, :], in_=raw[0:64, :])
                tmp = rp.tile([128, CH], F32, tag="tmp", name="tmp")
                nc.vector.tensor_mul(tmp[:], shf[:], sinc[:])
                nc.vector.tensor_mul(dst_ap, raw[:], cosc[:])
                nc.vector.tensor_add(dst_ap, dst_ap, tmp[:])

            for c in range(NCH):
                # ---- projections for chunk c: Q, K, V passes ----
                cosc = csp.tile([128, CH], F32, tag="cs", name="cosc")
                sinc = csp.tile([128, CH], F32, tag="cs", name="sinc")
                nc.sync.dma_start(out=cosc[:], in_=cosT[:, bass.ts(c, CH)])
                nc.sync.dma_start(out=sinc[:], in_=sinTs[:, bass.ts(c, CH)])
                hts = []
                QTc = [qtp.tile([128, CH], F32R, tag="qtc", name=f"QTc{i}")
                       for i in range(HPC)]
                qps = [psp.tile([128, CH], F32, tag="ps", name=f"qps{i}")
                       for i in range(HPC)]
                for k in range(KT_TILES):
                    ht_t = htp.tile([128, CH], PDT, tag="ht", name="ht_t")
                    nc.sync.dma_start(
                        out=ht_t[:], in_=hT[bass.ts(k, 128), bass.ts(c, CH)])
                    hts.append(ht_t)
                    wq_t = wp.tile([128, HC], PDT, tag="w", name="wq_t")
                    nc.sync.dma_start(out=wq_t[:], in_=wq[bass.ts(k, 128), :])
                    st, sp = (k == 0), (k == KT_TILES - 1)
                    for d in range(HPC):
                        mm(qps[d][:], wq_t[:, bass.ts(d, 128)], ht_t[:], st, sp)
                for d in range(HPC):
                    rope_evict(qps[d], QTc[d][:], cosc, sinc)
                kps = [psp.tile([128, CH], F32, tag="ps", name=f"kps{i}")
                       for i in range(HPC)]
                for k in range(KT_TILES):
                    wk_t = wp.tile([128, HC], PDT, tag="w", name="wk_t")
                    nc.sync.dma_start(out=wk_t[:], in_=wk[bass.ts(k, 128), :])
                    st, sp = (k == 0), (k == KT_TILES - 1)
                    for d in range(HPC):
                        mm(kps[d][:], wk_t[:, bass.ts(d, 128)], hts[k][:],
                           st, sp)
                for d in range(HPC):
                    rope_evict(kps[d], KT[d][:, bass.ts(c, CH)], cosc, sinc)
                vps = [psp.tile([128, HC], F32, tag="ps", name=f"vps{i}")
                       for i in range(HPC)]
                for k in range(KT_TILES):
                    wv_t = wp.tile([128, HC], PDT, tag="w", name="wv_t")
                    nc.sync.dma_start(out=wv_t[:], in_=wv[bass.ts(k, 128), :])
                    st, sp = (k == 0), (k == KT_TILES - 1)
                    for jl in range(4):
                        mm(vps[jl][:], hts[k][:, bass.ts(jl, 128)], wv_t[:],
                           st, sp)
                for jl in range(4):
                    nc.scalar.copy(out=V[4 * c + jl][:], in_=vps[jl][:])

                # ---- attention for i-chunk c (needs K/V chunks <= c) ----
                ic = c
                jmax = 4 * ic + 4 if causal else SJT
                for h in range(HPC):
                    sum_ps = psp.tile([1, CH], F32, tag="sum", bufs=1,
                                      name="sum_ps")
                    o_ps = psp.tile([128, CH], F32, tag="o", bufs=1,
                                    name="o_ps")
                    for j in range(jmax):
                        s_ps = psp.tile([128, CH], F32, tag="ps", name="s_ps")
                        mm(s_ps[:], KT[h][:, bass.ts(j, 128)],
                           QTc[h][:], True, True)
                        if causal:
                            if j >= 4 * ic:
                                nc.vector.tensor_add(
                                    s_ps[:], s_ps[:], mtiles[j - 4 * ic][:])
                        else:
                            mt = mpool.tile([128, CH], F32, tag="mt",
                                            name="mt")
                            nc.sync.dma_start(
                                out=mt[:],
                                in_=maskT[bass.ts(j, 128), bass.ts(ic, CH)])
                            nc.vector.tensor_add(s_ps[:], s_ps[:], mt[:])
                        es_t = esp.tile([128, CH], F32R, tag="es", name="es_t")
                        nc.scalar.activation(es_t[:], s_ps[:], EXPF)
                        st, sp = (j == 0), (j == jmax - 1)
                        mm(sum_ps[:], ones_col[:], es_t[:], st, sp)
                        mm(o_ps[:], V[j][:, bass.ts(h, 128)], es_t[:], st, sp)
                    rsum = rpool.tile([1, CH], F32R, tag="rs", name="rsum")
                    with nc.allow_low_precision(reason="f32r softmax recip"):
                        nc.vector.reciprocal(rsum[:], sum_ps[:])
                    b_ps = psp.tile([128, CH], F32, tag="ps", name="b_ps")
                    mm(b_ps[:], ones_row[:], rsum[:], True, True)
                    rb = rpool.tile([128, CH], F32, tag="rb", name="rb")
                    nc.scalar.copy(out=rb[:], in_=b_ps[:])
                    att = atsp.tile([128, CH], F32R, tag="att", name="att")
                    nc.vector.tensor_mul(att[:], o_ps[:], rb[:])
                    nc.sync.dma_start(
                        out=atb[bass.ts(h, 128), bass.ts(ic, CH)], in_=att[:])
        es_res.close()   # free KT/V SBUF before o_proj

        # ---------- o_proj  po = wo^T @ attnT ----------
        with tc.tile_pool(name="o_wo", bufs=HPC) as wop, \
             tc.tile_pool(name="o_at", bufs=6) as atp, \
             tc.tile_pool(name="o_out", bufs=4) as outp, \
             tc.tile_pool(name="o_ps", bufs=4, space="PSUM") as psp:
            WO = [wop.tile([128, H], F32R, tag="wo", name=f"WO{i}")
                  for i in range(HPC)]
            for kl in range(HPC):
                nc.sync.dma_start(out=WO[kl][:], in_=wo[bass.ts(kl, 128), :])
            for ic in range(NCH):
                ats = []
                for kl in range(HPC):
                    at_t = atp.tile([128, CH], F32R, tag="at", name="at_t")
                    nc.sync.dma_start(
                        out=at_t[:],
                        in_=atb[bass.ts(kl, 128), bass.ts(ic, CH)])
                    ats.append(at_t)
                for n in range(H // 128):
                    pps = psp.tile([128, CH], F32, tag="ps", name="pps")
                    for kl in range(HPC):
                        mm(pps[:], WO[kl][:, bass.ts(n, 128)], ats[kl][:],
                           kl == 0, kl == HPC - 1)
                    ot = outp.tile([128, CH], F32, tag="ot", name="ot")
                    nc.scalar.copy(out=ot[:], in_=pps[:])
                    nc.sync.dma_start(
                        out=po[bass.ts(n, 128), bass.ts(ic, CH)], in_=ot[:])
    nc.compile()
    return nc


_CACHE = {}


def _get_nc(causal, proj_bf16):
    key = (causal, proj_bf16)
    if key not in _CACHE:
        _CACHE[key] = _build(causal, proj_bf16)
    return _CACHE[key]


def kernel(hidden_states, attention_mask, position_ids, Wq, Wk, Wv, Wo):
    global last_exec_time_ns
    B, S_, H_ = hidden_states.shape
    assert (B, S_, H_) == (1, S, H)
    hs = np.asarray(hidden_states, dtype=np.float32)
    mask = np.asarray(attention_mask, dtype=np.float32)[0, 0]
    pos = np.asarray(position_ids)[0].astype(np.float64)

    # causal-mask fast path check: 0 on/below diagonal, very-negative above
    iu = np.triu_indices(S, k=1)
    il = np.tril_indices(S, k=0)
    causal = bool(np.all(mask[il] == 0.0) and np.all(mask[iu] <= -1e30))

    hT = _r(np.asarray(hs[0]).T)
    scale = 1.0 / np.sqrt(D)

    inv_freq = 1.0 / (ROPE_BASE ** (np.arange(0, D, 2, dtype=np.float64) / D))
    ang = pos[None, :] * np.concatenate([inv_freq, inv_freq])[:, None]  # [D,S]
    cosT = _r(np.cos(ang))
    sgn = np.ones((D, 1)); sgn[: D // 2] = -1.0
    sinTs = _r(np.sin(ang) * sgn)

    if causal:
        mp = np.zeros((4, 128, CH), dtype=np.float32)
        jj = np.arange(128)[:, None]
        ii = np.arange(CH)[None, :]
        for p in range(4):
            mp[p][(p * 128 + jj) > ii] = NEG
        mp = _r(mp)
    else:
        maskT = _r(mask.T)

    proj_bf16 = bool(int(os.environ.get("BASS_PROJ_BF16", "0")))
    if proj_bf16:
        import ml_dtypes
        _p = lambda x: np.ascontiguousarray(np.asarray(x, np.float32),
                                            dtype=ml_dtypes.bfloat16)
    else:
        _p = _r
    nc = _get_nc(causal, proj_bf16)
    in_maps = []
    for c in range(N_CORES):
        sl = slice(c * HC, (c + 1) * HC)
        m = {
            "hT": _p(hT),
            "wq": _p(np.asarray(Wq, np.float64)[:, sl] * scale),
            "wk": _p(np.asarray(Wk)[:, sl]),
            "wv": _p(np.asarray(Wv)[:, sl]),
            "wo": _r(np.asarray(Wo)[sl, :]),
            "cosT": cosT,
            "sinTs": sinTs,
        }
        if causal:
            m["mpat"] = mp
        else:
            m["maskT"] = maskT
        in_maps.append(m)

    trace = bool(int(os.environ.get("BASS_KERNEL_TRACE", "0")))
    kw = {}
    if trace:
        kw["trace"] = True
        kw["tmpdir"] = os.environ.get("BASS_KERNEL_TRACE_DIR") or None
    res = run_bass_kernel_spmd(nc, in_maps, list(range(N_CORES)), **kw)
    last_exec_time_ns = res.exec_time_ns

    acc = np.zeros((H, S), dtype=np.float64)
    for c in range(N_CORES):
        acc += res.results[c]["po"]
    out = acc.T.astype(np.float32).reshape(1, S, H)
    return out

